# revision 9
# baseline (speedup 1.0000x reference)
"""TRN2 Bass kernel for nn_ClusterNet: soft k-means over [500000,128] embeddings.

Sharding: N split across 8 NeuronCores; mu tiny + replicated; per-iteration
cluster stats AllReduce'd (per spec sharding_hint).

Per-core design (n_shard = 62500 rows):
  - dn (row-normalized embeddings) SBUF-resident in TWO layouts:
      dnT [D=128 part, n free] fp16   -> comp1 weights (cos = dn @ mn.T)
      dn8 [n part, D+1 free]  fp8e4m3 (x64, aug col = 64) -> comp2 moving
  - 11 iterations, each:
      comp1: per 128-row tile: LDW(dnT tile, FWL) + MM(rhs=mnT[128,16])
             -> psum cos [n,16]
      softmax: ACT exp (scale=-0.5; |logits|<=0.5 so no max-sub), DVE
               sum/recip/scale -> r8 = fp8(16*r)
      comp2: MM(lhsT=r8 tile [n,16], rhs=dn8 tile [n,129]) accumulating
             psum stats [16,129] = 1024*(r.T@[dn|1]) over all tiles
      AllReduce stats; mu = stats_mean/stats_r (1024 scale cancels)
  - last iteration also writes r (f32) and mu (f32) to HBM.
"""
import sys
import numpy as np


def _ensure_paths():
    try:
        import concourse.bass  # noqa: F401
    except ImportError:
        for p in [
            "/root/.axon_site",
            "/root/.axon_site/_ro/trn_rl_repo",
            "/root/.axon_site/_ro/pypackages",
            "/opt/trn_rl_repo",
        ]:
            if p not in sys.path:
                sys.path.insert(0, p)


_ensure_paths()

import concourse.bass as bass  # noqa: E402,F401
import concourse.bacc as bacc  # noqa: E402
import concourse.mybir as mybir  # noqa: E402
import concourse.tile as tile  # noqa: E402
from concourse import bass_utils, masks  # noqa: E402

F32 = mybir.dt.float32
C1 = mybir.dt.float16     # comp1 dtype (dnT, mnT, transposes)
C2 = mybir.dt.float8e4    # comp2 dtype (dn8 aug, r8)

N_FULL, D, K = 500000, 128, 16
NCORES = 8
NITER = 11
TAU = 0.5
DN_SCALE = 64.0           # dn8 = fp8(64*dn); aug col = 64.0
DSCALE = 256.0            # delta8 = fp8(256*r - 16); variance-reduced comp2
# comp2 accumulates S = sum_n (256r-16)*[64dn|64] = 16384*T - 16*B where
# T = sum r*[dn|1] (what we want) and B = sum [64dn|64] (iteration-invariant,
# computed once at setup + AllReduce'd). mu = (S+16B)[:, :D] / (S+16B)[:, D].
AUG = D + 1               # 129
SET_T = 8                 # tiles per setup group (1024 rows)
IG = 32                   # tiles per iteration group (psum bank = 512 f32)
AX = mybir.AxisListType
AF = mybir.ActivationFunctionType
ALU = mybir.AluOpType


def build_nc(n_shard):
    nt = (n_shard + 127) // 128              # number of 128-row tiles
    tile_rows = [128] * (n_shard // 128)
    if n_shard % 128:
        tile_rows.append(n_shard % 128)
    nfullg = n_shard // (SET_T * 128)        # full setup groups
    set_rows = SET_T * 128
    nig = (nt + IG - 1) // IG

    nc = bacc.Bacc("TRN2", target_bir_lowering=False, debug=False,
                   num_devices=NCORES)
    x_ap = nc.dram_tensor("x", [n_shard, D], F32, kind="ExternalInput").ap()
    mu0_ap = nc.dram_tensor("mu0", [K, D], F32, kind="ExternalInput").ap()
    mu_out = nc.dram_tensor("mu", [K, D], F32, kind="ExternalOutput").ap()
    r_out = nc.dram_tensor("r", [n_shard, K], F32, kind="ExternalOutput").ap()

    with tile.TileContext(nc) as tc:
        _body(tc, n_shard, nt, tile_rows, nfullg, set_rows, nig,
              x_ap, mu0_ap, mu_out, r_out)
    nc.compile()
    return nc


def _body(tc, n_shard, nt, tile_rows, nfullg, set_rows, nig,
          x_ap, mu0_ap, mu_out, r_out):
    nc = tc.nc

    with tc.tile_pool(name="persist", bufs=1) as persist:
        dnT = persist.tile([128, nt * 128], C1)
        dn8 = persist.tile([128, nt * AUG], C2)
        ident = persist.tile([128, 128], C1)
        ones8 = persist.tile([128, 1], C2)
        base_k = persist.tile([K, AUG], F32)

        masks.make_identity(nc, ident[:])
        nc.vector.memset(ones8[:], 1.0)
        # aug columns: every 129th col = DN_SCALE
        aug_cols = dn8[:].rearrange("p (t a) -> p t a", a=AUG)[:, :, D:D + 1]
        nc.vector.memset(aug_cols, DN_SCALE)

        _setup(tc, n_shard, nt, tile_rows, nfullg, set_rows, x_ap, dnT, dn8,
               ident, ones8, base_k)
        _iterate(tc, n_shard, nt, tile_rows, nfullg, set_rows, nig,
                 mu0_ap, mu_out, r_out, dnT, dn8, ident, base_k)


def _setup(tc, n_shard, nt, tile_rows, nfullg, set_rows, x_ap, dnT, dn8,
           ident, ones8, base_k):
    """Normalize rows; build dnT (fp16, transposed) + dn8 (fp8 x64, aug)."""
    nc = tc.nc
    with (
        tc.tile_pool(name="eb", bufs=2) as ebp,
        tc.tile_pool(name="sq", bufs=2) as sqp,
        tc.tile_pool(name="sstats", bufs=3) as ssp,
        tc.tile_pool(name="dnb", bufs=2) as dnbp,
        tc.tile_pool(name="pt", bufs=2, space="PSUM") as ptp,
        tc.tile_pool(name="pbase", bufs=1, space="PSUM") as pbp,
        tc.tile_pool(name="dramb", bufs=1, space="DRAM") as drbp,
    ):
        pbase = pbp.tile([1, AUG], F32)
        def setup_tiles(ebuf, rows_list, ct0):
            t_cnt = len(rows_list)
            w = t_cnt * 128
            sq = sqp.tile([128, set_rows], F32, name=f"sq_{ct0}", tag="sq")
            nc.scalar.square(sq[:, :w], ebuf[:, :w])
            ss = ssp.tile([128, SET_T], F32, name=f"ss_{ct0}", tag="ss")
            nc.vector.reduce_sum(
                ss[:, :t_cnt],
                sq[:, :w].rearrange("p (t d) -> p t d", d=128), axis=AX.X)
            sr = ssp.tile([128, SET_T], F32, name=f"sr_{ct0}", tag="sr")
            nc.scalar.sqrt(sr[:, :t_cnt], ss[:, :t_cnt])
            inv = ssp.tile([128, SET_T], F32, name=f"inv_{ct0}", tag="inv")
            nc.vector.reciprocal(inv[:, :t_cnt], sr[:, :t_cnt])
            inv_b = inv[:, :t_cnt, None].to_broadcast((128, t_cnt, 128))

            dnb = dnbp.tile([128, set_rows], C1, name=f"dnb_{ct0}", tag="dnb")
            nc.vector.tensor_tensor(
                dnb[:, :w].rearrange("p (t d) -> p t d", d=128),
                ebuf[:, :w].rearrange("p (t d) -> p t d", d=128),
                inv_b, ALU.mult)
            dst8 = dn8[:].rearrange("p (t a) -> p t a", a=AUG)[
                :, ct0:ct0 + t_cnt, 0:D]
            nc.vector.scalar_tensor_tensor(
                dst8,
                ebuf[:, :w].rearrange("p (t d) -> p t d", d=128),
                DN_SCALE, inv_b, ALU.mult, ALU.mult)
            pt = ptp.tile([128, set_rows], C1, name=f"pt_{ct0}", tag="pt")
            for t in range(t_cnt):
                rows = rows_list[t]
                nc.tensor.transpose(
                    pt[:, t * 128:t * 128 + rows],
                    dnb[:rows, t * 128:(t + 1) * 128],
                    ident[:rows, :rows])
            nc.vector.tensor_copy(dnT[:, ct0 * 128:ct0 * 128 + w], pt[:, :w])

        for g in range(nfullg):
            ebuf = ebp.tile([128, set_rows], F32, name=f"eb_{g}", tag="eb")
            src = x_ap[g * set_rows:(g + 1) * set_rows, :].rearrange(
                "(p t) d -> p (t d)", p=128)
            nc.sync.dma_start(ebuf[:], src)
            setup_tiles(ebuf, [128] * SET_T, g * SET_T)

        off = nfullg * set_rows
        ct = nfullg * SET_T
        while off < n_shard:
            rows = min(128, n_shard - off)
            ebuf = ebp.tile([128, set_rows], F32, name=f"eb_t{ct}", tag="eb")
            nc.sync.dma_start(ebuf[:rows, :D], x_ap[off:off + rows, :])
            setup_tiles(ebuf, [rows], ct)
            off += rows
            ct += 1

        # base B = sum_n [64dn|64]: one contiguous accumulation pass
        for ct in range(nt):
            vp = tile_rows[ct]
            nc.tensor.matmul(
                pbase[:],
                lhsT=ones8[:vp, :],
                rhs=dn8[:vp, ct * AUG:(ct + 1) * AUG],
                start=(ct == 0), stop=(ct == nt - 1),
                skip_group_check=True)
        # AllReduce the base across cores; broadcast to K partitions
        b_in = drbp.tile([1, AUG], F32, name="base_in")
        b_out = drbp.tile([1, AUG], F32, name="base_out", addr_space="Shared")
        b_pre = ssp.tile([1, AUG], F32, name="base_pre", tag="bpre")
        nc.vector.tensor_copy(b_pre[:], pbase[:])
        nc.sync.dma_start(b_in[:], b_pre[:])
        nc.gpsimd.collective_compute(
            "AllReduce", ALU.add,
            replica_groups=[list(range(NCORES))],
            ins=[b_in.opt()], outs=[b_out.opt()])
        b_out_ap = b_out[:]
        b_bcast = bass.AP(tensor=b_out_ap.tensor, offset=b_out_ap.offset,
                          ap=[[0, K]] + list(b_out_ap.ap[1:]))
        nc.sync.dma_start(base_k[:], b_bcast)


def _iterate(tc, n_shard, nt, tile_rows, nfullg, set_rows, nig,
             mu0_ap, mu_out, r_out, dnT, dn8, ident, base_k):
    nc = tc.nc
    with (
        tc.tile_pool(name="mus", bufs=2) as mup,
        tc.tile_pool(name="mnsmall", bufs=2) as mnp,
        tc.tile_pool(name="pa", bufs=2, space="PSUM") as pap,
        tc.tile_pool(name="pstats", bufs=2, space="PSUM") as pstp,
        tc.tile_pool(name="pmn", bufs=2, space="PSUM") as pmnp,
        tc.tile_pool(name="ebv", bufs=3) as ebvp,
        tc.tile_pool(name="sums", bufs=3) as sump,
        tc.tile_pool(name="r8", bufs=3) as r8p,
        tc.tile_pool(name="t256", bufs=2) as t256p,
        tc.tile_pool(name="rf", bufs=2) as rfp,
        tc.tile_pool(name="stats", bufs=2) as statp,
        tc.tile_pool(name="dram", bufs=2, space="DRAM") as dramp,
    ):
        mu_sb = mup.tile([K, D], F32, name="mu_it0", tag="mu")
        nc.sync.dma_start(mu_sb[:], mu0_ap)

        for it in range(NITER):
            last = it == NITER - 1
            # ---- mn = normalize(mu); mnT fp16 [128, K]
            sqm = mnp.tile([K, D], F32, name=f"sqm_{it}", tag="sqm")
            nc.scalar.square(sqm[:], mu_sb[:])
            ssm = mnp.tile([K, 1], F32, name=f"ssm_{it}", tag="ssm")
            nc.vector.reduce_sum(ssm[:], sqm[:], axis=AX.X)
            srm = mnp.tile([K, 1], F32, name=f"srm_{it}", tag="srm")
            nc.scalar.sqrt(srm[:], ssm[:])
            sim = mnp.tile([K, 1], F32, name=f"sim_{it}", tag="sim")
            nc.vector.reciprocal(sim[:], srm[:])
            mn16 = mnp.tile([K, D], C1, name=f"mn16_{it}", tag="mn16")
            nc.vector.tensor_scalar_mul(mn16[:], mu_sb[:], sim[:])
            pmn = pmnp.tile([128, K], C1, name=f"pmn_{it}", tag="pmn")
            nc.tensor.transpose(pmn[:, :], mn16[:], ident[:K, :K])
            mnT = mnp.tile([128, K], C1, name=f"mnT_{it}", tag="mnT")
            nc.vector.tensor_copy(mnT[:], pmn[:])

            pstats = pstp.tile([K, AUG], F32, name=f"pstats_{it}",
                               tag="pstats")

            for ig in range(nig):
                ct0 = ig * IG
                gsz = min(IG, nt - ct0)
                ncols = gsz * K
                pa = pap.tile([128, IG * K], F32, name=f"pa_{it}_{ig}",
                              tag="pa")
                for j in range(gsz):
                    ct = ct0 + j
                    vp = tile_rows[ct]
                    nc.tensor.matmul(
                        pa[:vp, j * K:(j + 1) * K],
                        lhsT=dnT[:, ct * 128:ct * 128 + vp],
                        rhs=mnT[:],
                        start=True, stop=True, skip_group_check=True)
                ebv = ebvp.tile([128, IG * K], F32, name=f"ebv_{it}_{ig}",
                                tag="ebv")
                nc.scalar.activation(ebv[:, :ncols], pa[:, :ncols], AF.Exp,
                                     scale=-TAU)
                st = sump.tile([128, IG], F32, name=f"st_{it}_{ig}", tag="st")
                nc.vector.reduce_sum(
                    st[:, :gsz],
                    ebv[:, :ncols].rearrange("p (t k) -> p t k", k=K),
                    axis=AX.X)
                rs = sump.tile([128, IG], F32, name=f"rs_{it}_{ig}", tag="rs")
                nc.vector.reciprocal(rs[:, :gsz], st[:, :gsz])
                rs_b = rs[:, :gsz, None].to_broadcast((128, gsz, K))
                t256 = t256p.tile([128, IG * K], F32, name=f"t256_{it}_{ig}",
                                  tag="t256")
                nc.vector.scalar_tensor_tensor(
                    t256[:, :ncols].rearrange("p (t k) -> p t k", k=K),
                    ebv[:, :ncols].rearrange("p (t k) -> p t k", k=K),
                    DSCALE, rs_b, ALU.mult, ALU.mult)
                r8 = r8p.tile([128, IG * K], C2, name=f"r8_{it}_{ig}",
                              tag="r8")
                nc.gpsimd.tensor_scalar_add(r8[:, :ncols], t256[:, :ncols],
                                            -DSCALE / 16.0)
                for j in range(gsz):
                    ct = ct0 + j
                    vp = tile_rows[ct]
                    nc.tensor.matmul(
                        pstats[:],
                        lhsT=r8[:vp, j * K:(j + 1) * K],
                        rhs=dn8[:vp, ct * AUG:(ct + 1) * AUG],
                        start=(ct == 0), stop=(ct == nt - 1),
                        skip_group_check=True)
                if last:
                    rf = rfp.tile([128, IG * K], F32, name=f"rf_{ig}",
                                  tag="rf")
                    nc.vector.tensor_tensor(
                        rf[:, :ncols].rearrange("p (t k) -> p t k", k=K),
                        ebv[:, :ncols].rearrange("p (t k) -> p t k", k=K),
                        rs_b, ALU.mult)
                    sg0 = ct0 // SET_T
                    for sg in range(sg0, min(sg0 + IG // SET_T, nfullg)):
                        cols = (sg * SET_T - ct0) * K
                        dst = r_out[sg * set_rows:(sg + 1) * set_rows, :]
                        dst = dst.rearrange("(p t) k -> p (t k)", p=128)
                        nc.sync.dma_start(dst, rf[:, cols:cols + SET_T * K])
                    for j in range(gsz):
                        ct = ct0 + j
                        if ct < nfullg * SET_T:
                            continue
                        vp = tile_rows[ct]
                        roff = nfullg * set_rows + (ct - nfullg * SET_T) * 128
                        nc.sync.dma_start(
                            r_out[roff:roff + vp, :],
                            rf[:vp, j * K:(j + 1) * K])

            # ---- AllReduce stats, then mu update
            st_in = dramp.tile([K, AUG], F32, name=f"arin_{it}", tag="arin")
            st_out = dramp.tile([K, AUG], F32, name=f"arout_{it}",
                                tag="arout", addr_space="Shared")
            stats_pre = statp.tile([K, AUG], F32, name=f"statsp_{it}",
                                   tag="statsp")
            nc.vector.tensor_copy(stats_pre[:], pstats[:])
            nc.sync.dma_start(st_in[:], stats_pre[:])
            nc.gpsimd.collective_compute(
                "AllReduce", ALU.add,
                replica_groups=[list(range(NCORES))],
                ins=[st_in.opt()], outs=[st_out.opt()])
            stats_sb = statp.tile([K, AUG], F32, name=f"stats_{it}",
                                  tag="stats")
            nc.sync.dma_start(stats_sb[:], st_out[:])
            tot = statp.tile([K, AUG], F32, name=f"tot_{it}", tag="tot")
            nc.vector.scalar_tensor_tensor(
                tot[:], base_k[:], 16.0, stats_sb[:], ALU.mult, ALU.add)
            rc = statp.tile([K, 1], F32, name=f"rc_{it}", tag="rc")
            nc.vector.reciprocal(rc[:], tot[:, D:D + 1])
            mu_sb = mup.tile([K, D], F32, name=f"mu_{it + 1}", tag="mu")
            nc.vector.tensor_tensor(
                mu_sb[:], tot[:, 0:D],
                rc[:].to_broadcast((K, D)), ALU.mult)
            if last:
                nc.sync.dma_start(mu_out, mu_sb[:])


_NC_CACHE = {}


def _get_nc(n_shard):
    if n_shard not in _NC_CACHE:
        _NC_CACHE[n_shard] = build_nc(n_shard)
    return _NC_CACHE[n_shard]


def kernel(embeddings, mu_init):
    embeddings = np.ascontiguousarray(np.asarray(embeddings, np.float32))
    mu_init = np.ascontiguousarray(np.asarray(mu_init, np.float32))
    n = embeddings.shape[0]
    assert n % NCORES == 0
    n_shard = n // NCORES
    nc = _get_nc(n_shard)
    shards = np.split(embeddings, NCORES, axis=0)
    in_maps = [{"x": s, "mu0": mu_init} for s in shards]
    res = bass_utils.run_bass_kernel_spmd(nc, in_maps,
                                          core_ids=list(range(NCORES)))
    mu = res.results[0]["mu"]
    r = np.concatenate([res.results[c]["r"] for c in range(NCORES)], axis=0)
    return mu, r


if __name__ == "__main__":
    rng = np.random.default_rng(0)
    emb = rng.standard_normal((NCORES * 1024, D)).astype(np.float32)
    mu0 = rng.random((K, D)).astype(np.float32)
    mu, r = kernel(emb, mu0)
    print("mu", mu.shape, "r", r.shape, "rowsum", r.sum(axis=1)[:4])
